# revision 2
# baseline (speedup 1.0000x reference)
"""Trainium2 Bass kernel for a 7-layer binarized CNN (nn_MCNET).

Strategy: pure data parallel over 8 NeuronCores (8 images each). Per core,
each 3x3 VALID conv layer is computed as 9 accumulating matmuls (one per tap),
with the tap shift expressed as a free-dim offset into a flattened
[S*cin, rows*W] activation buffer. S row-bands of the image are stacked along
the partition dim (block-diagonal weights) so small channel counts still fill
the PE array. Layers 1-6 run in bf16 (exact: ternary activations x (+-1)
weights, fp32 PSUM accumulation); layer 0 runs fp32 on the raw input.
Raw Bass with explicit semaphores (standalone wait_ge instructions) because
this walrus build rejects instructions with >1 embedded sync wait.
"""
import sys, os, dataclasses
sys.path.insert(0, '/opt/trn_rl_repo')
import numpy as np

CH = [(3, 4), (4, 8), (8, 16), (16, 32), (32, 64), (64, 32), (32, 2)]
HIN = [256, 127, 125, 123, 121, 119, 117]
HOUT = [h - 2 for h in HIN]          # 254,125,123,121,119,117,115
S = [32, 16, 8, 4, 2, 2, 4]          # bands per layer
B = [8, 8, 16, 32, 64, 64, 29]       # band size (input rows, +2 halo stored)
CR = [2, 4, 4, 4, 4, 4, 4]           # psum-chunk rows (cr*Wout <= 512)
R = [8, 8, 16, 16, 16, 16, 16]       # out rows per band per psum tile
NIMG = 8
OB = 29                               # output band rows (115 = 4 bands of <=29)


def build_program():
    import concourse.bass as bass
    import concourse.mybir as mybir
    dt = mybir.dt
    AF = mybir.ActivationFunctionType

    nc = bass.Bass("TRN2", target_bir_lowering=False)
    x = nc.dram_tensor("x", (NIMG, 3, 256, 256), dt.float32, kind="ExternalInput")
    w0f = nc.dram_tensor("w0f", (96, 9 * 128), dt.float32, kind="ExternalInput")
    WBF_COLS = 9 * (128 * 4 + 64 + 8)  # L1..L6 M sizes: 128,128,128,128,64,8
    wbf = nc.dram_tensor("wbf", (128, WBF_COLS), dt.bfloat16, kind="ExternalInput")
    y = nc.dram_tensor("y", (NIMG, 2 * 115 * 115), dt.float32, kind="ExternalOutput")

    K = [S[l] * CH[l][0] for l in range(7)]   # 96,64,64,64,64,128,128
    M = [S[l] * CH[l][1] for l in range(7)]   # 128,128,128,128,128,64,8
    WOFF = [0]
    for l in range(1, 6):
        WOFF.append(WOFF[-1] + 9 * M[l])

    ctxs = []
    def alloc(cm):
        ctxs.append(cm)
        return cm.__enter__()

    WT0 = alloc(nc.sbuf_tensor("WT0", [128, 9 * 128], dt.float32))
    WTB = alloc(nc.sbuf_tensor("WTB", [128, WBF_COLS], dt.bfloat16))
    A0 = alloc(nc.sbuf_tensor("A0", [128, 2 * 10 * 256], dt.float32))   # 2 slots
    A = [None] * 7
    for l in range(1, 7):
        A[l] = alloc(nc.sbuf_tensor(f"A{l}", [128, (B[l] + 2) * HIN[l]], dt.bfloat16))
    T0 = alloc(nc.sbuf_tensor("T0", [128, 2048], dt.bfloat16))
    T1 = alloc(nc.sbuf_tensor("T1", [128, 4 * 254], dt.bfloat16))
    T3 = alloc(nc.sbuf_tensor("T3", [128, 4 * 127], dt.bfloat16))
    STG = [alloc(nc.sbuf_tensor(f"STG{i}", [128, 16 * 125], dt.bfloat16))
           for i in range(2)]
    OUTB = alloc(nc.sbuf_tensor("OUTB", [128, OB * 115], dt.float32))
    P = [alloc(nc.psum_tensor(f"P{i}", [128, 2048], dt.float32)) for i in range(2)]
    sem = {n: alloc(nc.semaphore(name=n)) for n in
           ['sdma', 'spe', 'sact', 'sdve', 'sgp']}

    # ---------------- plan walk ----------------
    # The walk is deterministic; each engine closure replays it, emitting only
    # its own ops. Counters give exact wait targets.
    def walk(E, me):
        cnt = {'dma': 0, 'pe': 0, 'act': 0, 'dve': 0, 'gp': 0}
        last_wait = {}

        def wait(eng, semn, val):
            if val <= 0:
                return
            k = (eng, semn)
            if last_wait.get(k, -1) >= val:
                return
            last_wait[k] = val
            if eng == me:
                E.wait_ge(sem[semn], val)

        def emit(eng, fn):
            # fn() emits one instruction and returns it (only when eng==me)
            if eng == me:
                return fn()
            return None

        def inc(inst, semn, v):
            if inst is not None:
                inst.then_inc(sem[semn], v)

        # --- init: memsets of activation buffers (NaN poisoning guard) ---
        memset_list = [(A0, 2 * 10 * 256), *[(A[l], (B[l] + 2) * HIN[l]) for l in range(1, 7)]]
        for buf, ncols in memset_list:
            i = emit('gp', lambda buf=buf, ncols=ncols: nc.gpsimd.memset(buf[0:128, 0:ncols], 0.0))
            cnt['gp'] += 1
            inc(i, 'sgp', 1)
        # --- weight DMAs ---
        for (dst, src) in ((WT0, w0f), (WTB, wbf)):
            i = emit('gp', lambda dst=dst, src=src: nc.gpsimd.dma_start(dst[0:src.shape[0], :], src[:]))
            cnt['dma'] += 1
            inc(i, 'sdma', 16)

        slot_free = [None, None]   # (sem_name, val) when psum slot was last freed
        stage_free = [None, None]  # (sem_name, val) when staging slot free
        t3_free = None             # dma count when T3 rebands of prev img done
        a0_free = [None, None]     # sem_pe val when A0 slot free
        l0_tile_pe = [0] * NIMG
        tile_g = 0

        for img in range(NIMG):
            # ---- input DMA (2 dmas: bands 0-30, band 31) ----
            aslot = img % 2
            off = aslot * 2560
            if a0_free[aslot] is not None:
                wait('gp', 'spe', a0_free[aslot])
            src_main = dataclasses.replace(
                x[img], ap=[[2048, 31], [65536, 3], [256, 10], [1, 256]])
            i = emit('gp', lambda src_main=src_main, off=off:
                     nc.gpsimd.dma_start(A0[0:93, off:off + 2560], src_main))
            cnt['dma'] += 1
            inc(i, 'sdma', 16)
            i = emit('gp', lambda img=img, off=off:
                     nc.gpsimd.dma_start(A0[93:96, off:off + 2048], x[img, :, 248:256, :]))
            cnt['dma'] += 1
            inc(i, 'sdma', 16)
            dma_in_done = cnt['dma']

            for l in range(7):
                cin, cout = CH[l]
                W, Wo, s_l, b_l, cr, r_l = HIN[l], HOUT[l], S[l], B[l], CR[l], R[l]
                ntile = -(-b_l // r_l)
                for t in range(ntile):
                    r0 = t * r_l
                    rows = min(r_l, b_l - r0)
                    nch = -(-rows // cr)
                    slot = tile_g % 2
                    PS = P[slot]
                    # ---- PE: waits ----
                    wait('pe', 'sgp', len(memset_list))
                    if l == 0:
                        wait('pe', 'sdma', 16 * dma_in_done)
                        if img == 0 and t == 0:
                            pass
                    else:
                        wait('pe', 'sdma', 16 * prev_ready)
                    if slot_free[slot] is not None:
                        wait('pe', slot_free[slot][0], slot_free[slot][1])
                    # ---- PE: matmuls (taps outer, chunks inner) ----
                    if l == 0:
                        Abuf, aoff = A0, aslot * 2560
                    else:
                        Abuf, aoff = A[l], 0
                    woff = 0 if l == 0 else WOFF[l - 1]
                    for tap in range(9):
                        ki, kj = tap // 3, tap % 3
                        lhsT = (WT0 if l == 0 else WTB)[0:K[l], woff + tap * M[l]: woff + tap * M[l] + M[l]]
                        for c in range(nch):
                            crr = min(cr, rows - c * cr)
                            rbase = aoff + (r0 + c * cr + ki) * W + kj
                            def mk(l=l, c=c, crr=crr, rbase=rbase, tap=tap, W=W, Wo=Wo,
                                   Abuf=Abuf, PS=PS, lhsT=lhsT):
                                rv = Abuf[0:K[l], rbase:rbase + (crr - 1) * W + Wo]
                                rv = dataclasses.replace(
                                    rv, ap=[rv.ap[0], [W, crr], [1, Wo]])
                                ov = PS[0:M[l], c * 512: c * 512 + crr * Wo]
                                return nc.tensor.matmul(ov, lhsT, rv,
                                                        start=(tap == 0), stop=(tap == 8))
                            i = emit('pe', mk)
                            if tap == 8 and c == nch - 1:
                                cnt['pe'] += 1
                                inc(i, 'spe', 1)
                    if l == 0:
                        l0_tile_pe[img] = cnt['pe']
                        a0_free[aslot] = cnt['pe']
                    my_pe = cnt['pe']

                    # ---- evacuation ----
                    if l == 0:
                        # ACT: sign(psum) -> T0 (bf16, psum-chunk layout);
                        # sign commutes with max, so pool after sign.
                        wait('act', 'spe', my_pe)
                        i = emit('act', lambda PS=PS: nc.scalar.activation(
                            T0[0:128, 0:2048], PS[0:128, 0:2048], AF.Sign))
                        cnt['act'] += 1
                        inc(i, 'sact', 1)
                        slot_free[slot] = ('sact', cnt['act'])
                        # DVE: vmax rows then hmax cols (SBUF bf16)
                        wait('dve', 'sact', cnt['act'])
                        if t3_free is not None:
                            wait('dve', 'sdma', 16 * t3_free)
                        def mkv():
                            v = T0[0:128, 0:2048]
                            a = dataclasses.replace(v, ap=[v.ap[0], [512, 4], [1, 254]])
                            b = dataclasses.replace(v, offset=v.offset + 254,
                                                    ap=[v.ap[0], [512, 4], [1, 254]])
                            d = T1[0:128, 0:4 * 254]
                            d = dataclasses.replace(d, ap=[d.ap[0], [254, 4], [1, 254]])
                            return nc.vector.tensor_max(d, a, b)
                        i = emit('dve', mkv)
                        cnt['dve'] += 1
                        inc(i, 'sdve', 1)
                        def mkh():
                            sv = T1[0:128, 0:4 * 254]
                            a = dataclasses.replace(sv, ap=[sv.ap[0], [254, 4], [2, 127]])
                            b = dataclasses.replace(sv, offset=sv.offset + 1,
                                                    ap=[sv.ap[0], [254, 4], [2, 127]])
                            d = T3[0:128, 0:4 * 127]
                            d = dataclasses.replace(d, ap=[d.ap[0], [127, 4], [1, 127]])
                            return nc.vector.tensor_max(d, a, b)
                        i = emit('dve', mkh)
                        cnt['dve'] += 1
                        inc(i, 'sdve', 1)
                        # gp: reband T3 -> A1 via SBUF->SBUF DMA (compute
                        # engines require 32-aligned partition bases; DMA not)
                        wait('gp', 'sdve', cnt['dve'])
                        H1 = 127
                        for s in range(32):
                            g0, g1 = 4 * s, min(4 * s + 4, H1)
                            if g1 <= g0:
                                continue
                            for sp in (s // 2 - 1, s // 2):
                                if sp < 0 or sp >= 16:
                                    continue
                                d0, d1 = 8 * sp, min(8 * sp + 10, H1)
                                a0r, a1r = max(g0, d0), min(g1, d1)
                                if a1r <= a0r:
                                    continue
                                def mkc(s=s, sp=sp, a0r=a0r, a1r=a1r):
                                    sv = T3[4 * s:4 * s + 4,
                                            (a0r - 4 * s) * 127:(a1r - 4 * s) * 127]
                                    dv = A[1][4 * sp:4 * sp + 4,
                                              (a0r - 8 * sp) * 127:(a1r - 8 * sp) * 127]
                                    return nc.gpsimd.dma_start(dv, sv)
                                i = emit('gp', mkc)
                                cnt['dma'] += 1
                                inc(i, 'sdma', 16)
                        t3_free = cnt['dma']
                        prev_ready = cnt['dma']
                    else:
                        # 1) ACT: Sign(psum) -> compact staging (or OUTB for l==6),
                        #    base-0 partition access (PSUM alignment rule)
                        wait('act', 'spe', my_pe)
                        sslot = tile_g % 2
                        if l < 6 and stage_free[sslot] is not None:
                            wait('act', 'sdma', 16 * stage_free[sslot])
                        if l == 6 and img >= 1:
                            wait('act', 'sdma', 16 * outdma_done)
                        Mp = max(32, M[l])
                        nfull = rows // cr
                        rem = rows - nfull * cr
                        DSTC = STG[sslot] if l < 6 else OUTB
                        dst_row0 = 0 if l < 6 else r0
                        if nfull > 0:
                            def mks(PS=PS, Mp=Mp, nfull=nfull, cr=cr, Wo=Wo,
                                    DSTC=DSTC, dst_row0=dst_row0):
                                sv = PS[0:Mp, 0:(nfull - 1) * 512 + cr * Wo]
                                sv = dataclasses.replace(
                                    sv, ap=[sv.ap[0], [512, nfull], [1, cr * Wo]])
                                dv = DSTC[0:Mp, dst_row0 * Wo:(dst_row0 + nfull * cr) * Wo]
                                dv = dataclasses.replace(
                                    dv, ap=[dv.ap[0], [cr * Wo, nfull], [1, cr * Wo]])
                                return nc.scalar.activation(dv, sv, AF.Sign)
                            i = emit('act', mks)
                            cnt['act'] += 1
                            inc(i, 'sact', 1)
                        if rem > 0:
                            def mksr(PS=PS, Mp=Mp, nfull=nfull, rem=rem, Wo=Wo,
                                     DSTC=DSTC, dst_row0=dst_row0, cr=cr):
                                sv = PS[0:Mp, nfull * 512:nfull * 512 + rem * Wo]
                                r0d = dst_row0 + nfull * cr
                                dv = DSTC[0:Mp, r0d * Wo:(r0d + rem) * Wo]
                                return nc.scalar.activation(dv, sv, AF.Sign)
                            i = emit('act', mksr)
                            cnt['act'] += 1
                            inc(i, 'sact', 1)
                        slot_free[slot] = ('sact', cnt['act'])
                        # 2) gp: reband staging -> A[l+1] (SBUF->SBUF DMA)
                        if l < 6:
                            wait('gp', 'sact', cnt['act'])
                            Hn = HOUT[l]
                            Sp, Bp = S[l + 1], B[l + 1]
                            for s in range(s_l):
                                g0 = s * b_l + r0
                                g1 = min(s * b_l + r0 + rows, min((s + 1) * b_l, Hn))
                                if g1 <= g0:
                                    continue
                                for sp in range(Sp):
                                    d0 = sp * Bp
                                    d1 = min(sp * Bp + Bp + 2, Hn)
                                    a0r, a1r = max(g0, d0), min(g1, d1)
                                    if a1r <= a0r:
                                        continue
                                    def mkr(l=l, s=s, sp=sp, a0r=a0r, a1r=a1r,
                                            d0=d0, g0=g0, r0=r0, b_l=b_l, Wo=Wo,
                                            cout=cout, sslot=sslot):
                                        lr0 = a0r - s * b_l - r0
                                        n = a1r - a0r
                                        sv = STG[sslot][s * cout:(s + 1) * cout,
                                                        lr0 * Wo:(lr0 + n) * Wo]
                                        dv = A[l + 1][sp * cout:(sp + 1) * cout,
                                                      (a0r - d0) * Wo:(a1r - d0) * Wo]
                                        return nc.gpsimd.dma_start(dv, sv)
                                    i = emit('gp', mkr)
                                    cnt['dma'] += 1
                                    inc(i, 'sdma', 16)
                            stage_free[sslot] = cnt['dma']
                    tile_g += 1
                # end tiles
                if 1 <= l < 6:
                    prev_ready = cnt['dma']
            # ---- output DMA (4 bands: 29,29,29,28 rows) ----
            wait('gp', 'sact', cnt['act'])
            def mko1(img=img):
                sv = OUTB[0:6, 0:OB * 115]
                dv = y[img, 0:3 * OB * 115]
                dv = dataclasses.replace(
                    dv, ap=[[OB * 115, 3], [13225, 2], [115, OB], [1, 115]],
                    offset=dv.offset)
                sv = dataclasses.replace(sv, ap=[sv.ap[0], [1, OB * 115]])
                return nc.gpsimd.dma_start(dv, sv)
            i = emit('gp', mko1)
            cnt['dma'] += 1
            inc(i, 'sdma', 16)
            def mko2(img=img):
                sv = OUTB[6:8, 0:28 * 115]
                dv = y[img, 0:1]
                dv = dataclasses.replace(
                    dv, offset=dv.offset + 3 * OB * 115,
                    ap=[[13225, 2], [115, 28], [1, 115]])
                return nc.gpsimd.dma_start(dv, sv)
            i = emit('gp', mko2)
            cnt['dma'] += 1
            inc(i, 'sdma', 16)
            outdma_done = cnt['dma']
        return cnt

    with nc.Block() as block:
        @block.tensor
        def _(E):
            walk(E, 'pe')

        @block.scalar
        def _(E):
            walk(E, 'act')

        @block.vector
        def _(E):
            walk(E, 'dve')

        @block.gpsimd
        def _(E):
            walk(E, 'gp')

    for cm in reversed(ctxs):
        cm.__exit__(None, None, None)
    return nc


def pack_weights(ws):
    """ws: list of 7 raw weight arrays (cout, cin, 3, 3). Returns (w0f, wbf)."""
    import ml_dtypes
    sws = [np.sign(w).astype(np.float32) for w in ws]
    K = [S[l] * CH[l][0] for l in range(7)]
    M = [S[l] * CH[l][1] for l in range(7)]
    w0f = np.zeros((96, 9 * 128), np.float32)
    for tap in range(9):
        ki, kj = tap // 3, tap % 3
        blk = sws[0][:, :, ki, kj].T  # (cin, cout)
        for s in range(S[0]):
            w0f[s * 3:s * 3 + 3, tap * 128 + s * 4: tap * 128 + s * 4 + 4] = blk
    WBF_COLS = 9 * (128 * 4 + 64 + 8)
    wbf = np.zeros((128, WBF_COLS), np.float32)
    off = 0
    for l in range(1, 7):
        cin, cout = CH[l]
        for tap in range(9):
            ki, kj = tap // 3, tap % 3
            blk = sws[l][:, :, ki, kj].T
            for s in range(S[l]):
                wbf[s * cin:(s + 1) * cin,
                    off + tap * M[l] + s * cout: off + tap * M[l] + (s + 1) * cout] = blk
        off += 9 * M[l]
    return w0f, wbf.astype(ml_dtypes.bfloat16)


LAST_RES = None


def kernel(**inputs):
    global LAST_RES
    from concourse.bass_utils import run_bass_kernel_spmd
    inp = np.asarray(inputs['inputs'], np.float32)
    ws = [np.asarray(inputs[f'w{i}']) for i in range(7)]
    w0f, wbf = pack_weights(ws)
    nc = build_program()
    in_maps = []
    for c in range(8):
        in_maps.append({'x': np.ascontiguousarray(inp[c * 8:(c + 1) * 8]),
                        'w0f': w0f, 'wbf': wbf})
    res = run_bass_kernel_spmd(nc, in_maps, core_ids=list(range(8)),
                               trace=bool(os.environ.get('KTRACE')))
    LAST_RES = res
    out = np.concatenate([res.results[c]['y'] for c in range(8)], axis=0)
    return out.astype(np.float32)



# revision 4
# speedup vs baseline: 2.7281x; 2.7281x over previous
"""Trainium2 Bass kernel for a 7-layer binarized CNN (nn_MCNET).

Strategy: pure data parallel over 8 NeuronCores (8 images each). Per core,
each 3x3 VALID conv layer is computed as 9 accumulating matmuls (one per tap),
with the tap shift expressed as a free-dim offset into a flattened
[S*cin, rows*W] activation buffer. S row-bands of the image are stacked along
the partition dim (block-diagonal weights) so small channel counts still fill
the PE array. Layers 1-6 run in bf16 (exact: ternary activations x (+-1)
weights, fp32 PSUM accumulation); layer 0 runs fp32 on the raw input.
Raw Bass with explicit semaphores (standalone wait_ge instructions) because
this walrus build rejects instructions with >1 embedded sync wait.
"""
import sys, os, dataclasses
sys.path.insert(0, '/opt/trn_rl_repo')
import numpy as np

CH = [(3, 4), (4, 8), (8, 16), (16, 32), (32, 64), (64, 32), (32, 2)]
HIN = [256, 127, 125, 123, 121, 119, 117]
HOUT = [h - 2 for h in HIN]          # 254,125,123,121,119,117,115
S = [32, 16, 8, 4, 2, 2, 4]          # bands per layer
B = [8, 8, 16, 32, 64, 64, 29]       # band size (input rows, +2 halo stored)
CR = [2, 4, 4, 4, 4, 4, 4]           # psum-chunk rows (cr*Wout <= 512)
R = [8, 8, 16, 16, 16, 16, 16]       # out rows per band per psum tile
NIMG = 8
OB = 29                               # output band rows (115 = 4 bands of <=29)


def build_program():
    import concourse.bass as bass
    import concourse.mybir as mybir
    dt = mybir.dt
    AF = mybir.ActivationFunctionType

    nc = bass.Bass("TRN2", target_bir_lowering=False)
    x = nc.dram_tensor("x", (NIMG, 3, 256, 256), dt.float32, kind="ExternalInput")
    w0f = nc.dram_tensor("w0f", (96, 9 * 128), dt.float32, kind="ExternalInput")
    WBF_COLS = 9 * (128 * 4 + 64 + 8)  # L1..L6 M sizes: 128,128,128,128,64,8
    wbf = nc.dram_tensor("wbf", (128, WBF_COLS), dt.bfloat16, kind="ExternalInput")
    y = nc.dram_tensor("y", (NIMG, 2 * 115 * 115), dt.float32, kind="ExternalOutput")

    K = [S[l] * CH[l][0] for l in range(7)]   # 96,64,64,64,64,128,128
    M = [S[l] * CH[l][1] for l in range(7)]   # 128,128,128,128,128,64,8
    WOFF = [0]
    for l in range(1, 6):
        WOFF.append(WOFF[-1] + 9 * M[l])

    ctxs = []
    def alloc(cm):
        ctxs.append(cm)
        return cm.__enter__()

    WT0 = alloc(nc.sbuf_tensor("WT0", [128, 9 * 128], dt.float32))
    WTB = alloc(nc.sbuf_tensor("WTB", [128, WBF_COLS], dt.bfloat16))
    A0 = alloc(nc.sbuf_tensor("A0", [128, 2 * 10 * 256], dt.float32))   # 2 slots
    A = [None] * 7
    for l in range(1, 7):
        A[l] = alloc(nc.sbuf_tensor(f"A{l}", [128, (B[l] + 2) * HIN[l]], dt.bfloat16))
    T0 = alloc(nc.sbuf_tensor("T0", [128, 2048], dt.bfloat16))
    T1 = alloc(nc.sbuf_tensor("T1", [128, 4 * 254], dt.bfloat16))
    T3 = alloc(nc.sbuf_tensor("T3", [128, 4 * 127], dt.bfloat16))
    STG = [alloc(nc.sbuf_tensor(f"STG{i}", [128, 16 * 125], dt.bfloat16))
           for i in range(2)]
    OUTB = alloc(nc.sbuf_tensor("OUTB", [128, OB * 115], dt.float32))
    P = [alloc(nc.psum_tensor(f"P{i}", [128, 2048], dt.float32)) for i in range(2)]
    sem = {n: alloc(nc.semaphore(name=n)) for n in
           ['sdma', 'spe', 'sact', 'sdve', 'sgp']}

    # ---------------- plan walk ----------------
    # The walk is deterministic; each engine closure replays it, emitting only
    # its own ops. Counters give exact wait targets.
    def walk(E, me):
        cnt = {'dma': 0, 'pe': 0, 'act': 0, 'dve': 0, 'gp': 0}
        last_wait = {}

        def wait(eng, semn, val):
            if val <= 0:
                return
            k = (eng, semn)
            if last_wait.get(k, -1) >= val:
                return
            last_wait[k] = val
            if eng == me:
                E.wait_ge(sem[semn], val)

        def emit(eng, fn):
            # fn() emits one instruction and returns it (only when eng==me)
            if eng == me:
                return fn()
            return None

        def inc(inst, semn, v):
            if inst is not None:
                inst.then_inc(sem[semn], v)

        # --- init: memsets of activation buffers (NaN poisoning guard) ---
        memset_list = [(A0, 2 * 10 * 256), *[(A[l], (B[l] + 2) * HIN[l]) for l in range(1, 7)]]
        for buf, ncols in memset_list:
            i = emit('gp', lambda buf=buf, ncols=ncols: nc.gpsimd.memset(buf[0:128, 0:ncols], 0.0))
            cnt['gp'] += 1
            inc(i, 'sgp', 1)
        # --- weight DMAs ---
        for (dst, src) in ((WT0, w0f), (WTB, wbf)):
            i = emit('gp', lambda dst=dst, src=src: nc.gpsimd.dma_start(dst[0:src.shape[0], :], src[:]))
            cnt['dma'] += 1
            inc(i, 'sdma', 16)

        slot_free = [None, None]   # (sem_name, val) when psum slot was last freed
        stage_free = [None, None]  # (sem_name, val) when staging slot free
        t3_free = None             # dma count when T3 rebands of prev img done
        a0_free = [None, None]     # sem_pe val when A0 slot free
        l0_tile_pe = [0] * NIMG
        tile_g = 0

        for img in range(NIMG):
            # ---- input DMA (2 dmas: bands 0-30, band 31) ----
            aslot = img % 2
            off = aslot * 2560
            if a0_free[aslot] is not None:
                wait('gp', 'spe', a0_free[aslot])
            src_main = dataclasses.replace(
                x[img], ap=[[2048, 31], [65536, 3], [256, 10], [1, 256]])
            i = emit('gp', lambda src_main=src_main, off=off:
                     nc.gpsimd.dma_start(A0[0:93, off:off + 2560], src_main))
            cnt['dma'] += 1
            inc(i, 'sdma', 16)
            i = emit('gp', lambda img=img, off=off:
                     nc.gpsimd.dma_start(A0[93:96, off:off + 2048], x[img, :, 248:256, :]))
            cnt['dma'] += 1
            inc(i, 'sdma', 16)
            dma_in_done = cnt['dma']

            for l in range(7):
                cin, cout = CH[l]
                W, Wo, s_l, b_l, cr, r_l = HIN[l], HOUT[l], S[l], B[l], CR[l], R[l]
                ntile = -(-b_l // r_l)
                for t in range(ntile):
                    r0 = t * r_l
                    rows = min(r_l, b_l - r0)
                    nch = -(-rows // cr)
                    slot = tile_g % 2
                    PS = P[slot]
                    # ---- PE: waits ----
                    wait('pe', 'sgp', len(memset_list))
                    if l == 0:
                        wait('pe', 'sdma', 16 * dma_in_done)
                        if img == 0 and t == 0:
                            pass
                    else:
                        wait('pe', 'sdma', 16 * prev_ready)
                    if slot_free[slot] is not None:
                        wait('pe', slot_free[slot][0], slot_free[slot][1])
                    # ---- PE: matmuls (taps outer, chunks inner) ----
                    if l == 0:
                        Abuf, aoff = A0, aslot * 2560
                    else:
                        Abuf, aoff = A[l], 0
                    woff = 0 if l == 0 else WOFF[l - 1]
                    for tap in range(9):
                        ki, kj = tap // 3, tap % 3
                        lhsT = (WT0 if l == 0 else WTB)[0:K[l], woff + tap * M[l]: woff + tap * M[l] + M[l]]
                        for c in range(nch):
                            crr = min(cr, rows - c * cr)
                            rbase = aoff + (r0 + c * cr + ki) * W + kj
                            def mk(l=l, c=c, crr=crr, rbase=rbase, tap=tap, W=W, Wo=Wo,
                                   Abuf=Abuf, PS=PS, lhsT=lhsT):
                                rv = Abuf[0:K[l], rbase:rbase + (crr - 1) * W + Wo]
                                rv = dataclasses.replace(
                                    rv, ap=[rv.ap[0], [W, crr], [1, Wo]])
                                ov = PS[0:M[l], c * 512: c * 512 + crr * Wo]
                                return nc.tensor.matmul(ov, lhsT, rv,
                                                        start=(tap == 0), stop=(tap == 8))
                            i = emit('pe', mk)
                            if tap == 8 and c == nch - 1:
                                cnt['pe'] += 1
                                inc(i, 'spe', 1)
                    if l == 0:
                        l0_tile_pe[img] = cnt['pe']
                        a0_free[aslot] = cnt['pe']
                    my_pe = cnt['pe']

                    # ---- evacuation ----
                    if l == 0:
                        # ACT: sign(psum) -> T0 (bf16, psum-chunk layout);
                        # sign commutes with max, so pool after sign.
                        wait('act', 'spe', my_pe)
                        i = emit('act', lambda PS=PS: nc.scalar.activation(
                            T0[0:128, 0:2048], PS[0:128, 0:2048], AF.Sign))
                        cnt['act'] += 1
                        inc(i, 'sact', 1)
                        slot_free[slot] = ('sact', cnt['act'])
                        # DVE: vmax rows then hmax cols (SBUF bf16)
                        wait('dve', 'sact', cnt['act'])
                        if t3_free is not None:
                            wait('dve', 'sdma', 16 * t3_free)
                        def mkv():
                            v = T0[0:128, 0:2048]
                            a = dataclasses.replace(v, ap=[v.ap[0], [512, 4], [1, 254]])
                            b = dataclasses.replace(v, offset=v.offset + 254,
                                                    ap=[v.ap[0], [512, 4], [1, 254]])
                            d = T1[0:128, 0:4 * 254]
                            d = dataclasses.replace(d, ap=[d.ap[0], [254, 4], [1, 254]])
                            return nc.vector.tensor_max(d, a, b)
                        i = emit('dve', mkv)
                        cnt['dve'] += 1
                        inc(i, 'sdve', 1)
                        def mkh():
                            sv = T1[0:128, 0:4 * 254]
                            a = dataclasses.replace(sv, ap=[sv.ap[0], [254, 4], [2, 127]])
                            b = dataclasses.replace(sv, offset=sv.offset + 1,
                                                    ap=[sv.ap[0], [254, 4], [2, 127]])
                            d = T3[0:128, 0:4 * 127]
                            d = dataclasses.replace(d, ap=[d.ap[0], [127, 4], [1, 127]])
                            return nc.vector.tensor_max(d, a, b)
                        i = emit('dve', mkh)
                        cnt['dve'] += 1
                        inc(i, 'sdve', 1)
                        # gp: reband T3 -> A1 via SBUF->SBUF DMA (compute
                        # engines require 32-aligned partition bases; DMA not)
                        wait('gp', 'sdve', cnt['dve'])
                        H1 = 127
                        for s in range(32):
                            g0, g1 = 4 * s, min(4 * s + 4, H1)
                            if g1 <= g0:
                                continue
                            for sp in (s // 2 - 1, s // 2):
                                if sp < 0 or sp >= 16:
                                    continue
                                d0, d1 = 8 * sp, min(8 * sp + 10, H1)
                                a0r, a1r = max(g0, d0), min(g1, d1)
                                if a1r <= a0r:
                                    continue
                                def mkc(s=s, sp=sp, a0r=a0r, a1r=a1r):
                                    sv = T3[4 * s:4 * s + 4,
                                            (a0r - 4 * s) * 127:(a1r - 4 * s) * 127]
                                    dv = A[1][4 * sp:4 * sp + 4,
                                              (a0r - 8 * sp) * 127:(a1r - 8 * sp) * 127]
                                    return nc.gpsimd.dma_start(dv, sv)
                                i = emit('gp', mkc)
                                cnt['dma'] += 1
                                inc(i, 'sdma', 16)
                        t3_free = cnt['dma']
                        prev_ready = cnt['dma']
                    else:
                        # 1) ACT: Sign(psum) -> compact staging (or OUTB for l==6),
                        #    base-0 partition access (PSUM alignment rule)
                        wait('act', 'spe', my_pe)
                        sslot = tile_g % 2
                        if l < 6 and stage_free[sslot] is not None:
                            wait('act', 'sdma', 16 * stage_free[sslot])
                        if l == 6 and img >= 1:
                            wait('act', 'sdma', 16 * outdma_done)
                        Mp = max(32, M[l])
                        nfull = rows // cr
                        rem = rows - nfull * cr
                        DSTC = STG[sslot] if l < 6 else OUTB
                        dst_row0 = 0 if l < 6 else r0
                        if nfull > 0:
                            def mks(PS=PS, Mp=Mp, nfull=nfull, cr=cr, Wo=Wo,
                                    DSTC=DSTC, dst_row0=dst_row0):
                                sv = PS[0:Mp, 0:(nfull - 1) * 512 + cr * Wo]
                                sv = dataclasses.replace(
                                    sv, ap=[sv.ap[0], [512, nfull], [1, cr * Wo]])
                                dv = DSTC[0:Mp, dst_row0 * Wo:(dst_row0 + nfull * cr) * Wo]
                                dv = dataclasses.replace(
                                    dv, ap=[dv.ap[0], [cr * Wo, nfull], [1, cr * Wo]])
                                return nc.scalar.activation(dv, sv, AF.Sign)
                            i = emit('act', mks)
                            cnt['act'] += 1
                            inc(i, 'sact', 1)
                        if rem > 0:
                            def mksr(PS=PS, Mp=Mp, nfull=nfull, rem=rem, Wo=Wo,
                                     DSTC=DSTC, dst_row0=dst_row0, cr=cr):
                                sv = PS[0:Mp, nfull * 512:nfull * 512 + rem * Wo]
                                r0d = dst_row0 + nfull * cr
                                dv = DSTC[0:Mp, r0d * Wo:(r0d + rem) * Wo]
                                return nc.scalar.activation(dv, sv, AF.Sign)
                            i = emit('act', mksr)
                            cnt['act'] += 1
                            inc(i, 'sact', 1)
                        slot_free[slot] = ('sact', cnt['act'])
                        # 2) gp: reband staging -> A[l+1] (SBUF->SBUF DMA)
                        if l < 6:
                            wait('gp', 'sact', cnt['act'])
                            Hn = HOUT[l]
                            Sp, Bp = S[l + 1], B[l + 1]
                            for s in range(s_l):
                                g0 = s * b_l + r0
                                g1 = min(s * b_l + r0 + rows, min((s + 1) * b_l, Hn))
                                if g1 <= g0:
                                    continue
                                for sp in range(Sp):
                                    d0 = sp * Bp
                                    d1 = min(sp * Bp + Bp + 2, Hn)
                                    a0r, a1r = max(g0, d0), min(g1, d1)
                                    if a1r <= a0r:
                                        continue
                                    def mkr(l=l, s=s, sp=sp, a0r=a0r, a1r=a1r,
                                            d0=d0, g0=g0, r0=r0, b_l=b_l, Wo=Wo,
                                            cout=cout, sslot=sslot):
                                        lr0 = a0r - s * b_l - r0
                                        n = a1r - a0r
                                        sv = STG[sslot][s * cout:(s + 1) * cout,
                                                        lr0 * Wo:(lr0 + n) * Wo]
                                        dv = A[l + 1][sp * cout:(sp + 1) * cout,
                                                      (a0r - d0) * Wo:(a1r - d0) * Wo]
                                        return nc.gpsimd.dma_start(dv, sv)
                                    i = emit('gp', mkr)
                                    cnt['dma'] += 1
                                    inc(i, 'sdma', 16)
                            stage_free[sslot] = cnt['dma']
                    tile_g += 1
                # end tiles
                if 1 <= l < 6:
                    prev_ready = cnt['dma']
            # ---- output DMA (4 bands: 29,29,29,28 rows) ----
            wait('gp', 'sact', cnt['act'])
            def mko1(img=img):
                sv = OUTB[0:6, 0:OB * 115]
                dv = y[img, 0:3 * OB * 115]
                dv = dataclasses.replace(
                    dv, ap=[[OB * 115, 3], [13225, 2], [115, OB], [1, 115]],
                    offset=dv.offset)
                sv = dataclasses.replace(sv, ap=[sv.ap[0], [1, OB * 115]])
                return nc.gpsimd.dma_start(dv, sv)
            i = emit('gp', mko1)
            cnt['dma'] += 1
            inc(i, 'sdma', 16)
            def mko2(img=img):
                sv = OUTB[6:8, 0:28 * 115]
                dv = y[img, 0:1]
                dv = dataclasses.replace(
                    dv, offset=dv.offset + 3 * OB * 115,
                    ap=[[13225, 2], [115, 28], [1, 115]])
                return nc.gpsimd.dma_start(dv, sv)
            i = emit('gp', mko2)
            cnt['dma'] += 1
            inc(i, 'sdma', 16)
            outdma_done = cnt['dma']
        return cnt

    with nc.Block() as block:
        @block.tensor
        def _(E):
            walk(E, 'pe')

        @block.scalar
        def _(E):
            walk(E, 'act')

        @block.vector
        def _(E):
            walk(E, 'dve')

        @block.gpsimd
        def _(E):
            walk(E, 'gp')

    for cm in reversed(ctxs):
        cm.__exit__(None, None, None)
    return nc


def pack_weights(ws):
    """ws: list of 7 raw weight arrays (cout, cin, 3, 3). Returns (w0f, wbf)."""
    import ml_dtypes
    sws = [np.sign(w).astype(np.float32) for w in ws]
    K = [S[l] * CH[l][0] for l in range(7)]
    M = [S[l] * CH[l][1] for l in range(7)]
    w0f = np.zeros((96, 9 * 128), np.float32)
    for tap in range(9):
        ki, kj = tap // 3, tap % 3
        blk = sws[0][:, :, ki, kj].T  # (cin, cout)
        for s in range(S[0]):
            w0f[s * 3:s * 3 + 3, tap * 128 + s * 4: tap * 128 + s * 4 + 4] = blk
    WBF_COLS = 9 * (128 * 4 + 64 + 8)
    wbf = np.zeros((128, WBF_COLS), np.float32)
    off = 0
    for l in range(1, 7):
        cin, cout = CH[l]
        for tap in range(9):
            ki, kj = tap // 3, tap % 3
            blk = sws[l][:, :, ki, kj].T
            for s in range(S[l]):
                wbf[s * cin:(s + 1) * cin,
                    off + tap * M[l] + s * cout: off + tap * M[l] + (s + 1) * cout] = blk
        off += 9 * M[l]
    return w0f, wbf.astype(ml_dtypes.bfloat16)


LAST_RES = None


def kernel(**inputs):
    global LAST_RES
    from concourse.bass_utils import run_bass_kernel_spmd
    inp = np.asarray(inputs['inputs'], np.float32)
    ws = [np.asarray(inputs[f'w{i}']) for i in range(7)]
    w0f, wbf = pack_weights(ws)
    nc = build_program()
    in_maps = []
    for c in range(8):
        in_maps.append({'x': np.ascontiguousarray(inp[c * 8:(c + 1) * 8]),
                        'w0f': w0f, 'wbf': wbf})
    res = run_bass_kernel_spmd(nc, in_maps, core_ids=list(range(8)),
                               trace=bool(os.environ.get('KTRACE')))
    LAST_RES = res
    out = np.concatenate([res.results[c]['y'] for c in range(8)], axis=0)
    return out.astype(np.float32)

